# revision 21
# baseline (speedup 1.0000x reference)
"""ConcatCritic all-pairs MLP scores on 8 Trainium2 NeuronCores.

scores[i, j] = MLP(concat(x[j], y[i])) computed as a [B, B] grid, sharded
by y-rows across 8 cores (each core computes a [B/8, B] slab).

Key restructure: layer 1 of the MLP acts on concat(x[j], y[i]), so
    z1[i, j, :] = x[j] @ W1x + (y[i] @ W1y + b1)
which is precomputed once as AT = (x @ W1x).T  [H, B] and
CT = (y_slab @ W1y + b1).T  [H, R].  Per row i, h1.T = relu(AT + CT[:, i])
is a single per-partition scalar add+max on the vector engine. This removes
the [B*B, 256] @ [256, 512] matmul entirely.

Layer 2 runs on the tensor engine in float32r (FP22 multiplies, FP32
accumulate) at 1 cycle/row. Layer 3 is restructured off the tensor engine:
    s[j] = sum_m w3[m] relu(z2[m,j] + b2[m]) + b3
        = sum_m sign(w3[m]) * t[m,j] + b3,   t = |w3| * relu(z2 + b2)
t comes out of the scalar-engine activation for free (scale=|w3|,
bias=|w3|*b2), the sign-weighted partition-block sum runs on the vector
engine (1 tensor_scalar + 3 fused scalar_tensor_tensor per row, bf16), and
a single ones-vector matmul (512 cycles instead of the 2048 the M=1 W3
matmuls took) reduces the 128 partitions.
"""

import threading

import numpy as np

B = 512
DX = 128
DY = 128
H = 512
P = 128
NCORES = 8
R = B // NCORES  # 64 rows of the pair grid per core
HB = H // P  # 4 partition-blocks of the hidden dim
JB = B // P  # 4 partition-blocks of the j axis
GS = 8  # output rows batched per store DMA

_cache_lock = threading.Lock()
_cached_nc = {}


def _build_bass(nloop=1):
    """Emit the Bass/Tile program for one core's [R, B] slab."""
    import concourse.bass as bass  # noqa: F401
    import concourse.tile as tile
    from concourse import bacc, mybir
    from concourse.masks import make_identity

    f32 = mybir.dt.float32
    f32r = mybir.dt.float32r
    bf16 = mybir.dt.bfloat16
    Relu = mybir.ActivationFunctionType.Relu
    Copy = mybir.ActivationFunctionType.Copy
    add = mybir.AluOpType.add
    amax = mybir.AluOpType.max
    mult = mybir.AluOpType.mult

    nc = bacc.Bacc(
        "TRN2",
        target_bir_lowering=False,
        debug=False,
        enable_asserts=False,
    )

    x_d = nc.dram_tensor("x", (B, DX), f32, kind="ExternalInput").ap()
    ys_d = nc.dram_tensor("ys", (R, DY), f32, kind="ExternalInput").ap()
    w1_d = nc.dram_tensor("w1", (DX + DY, H), f32r, kind="ExternalInput").ap()
    b1_d = nc.dram_tensor("b1", (H,), f32, kind="ExternalInput").ap()
    w2_d = nc.dram_tensor("w2", (H, H), f32r, kind="ExternalInput").ap()
    ascale_d = nc.dram_tensor("ascale", (H,), f32, kind="ExternalInput").ap()
    abias_d = nc.dram_tensor("abias", (H,), f32, kind="ExternalInput").ap()
    s3_d = nc.dram_tensor("s3", (H,), f32, kind="ExternalInput").ap()
    b3r_d = nc.dram_tensor("b3r", (P,), f32, kind="ExternalInput").ap()
    out_d = nc.dram_tensor("s_slab", (R, B), f32, kind="ExternalOutput").ap()

    with tile.TileContext(nc) as tc:
        with (
            tc.tile_pool(name="const", bufs=1) as cpool,
            tc.tile_pool(name="h1p", bufs=3) as h1pool,
            tc.tile_pool(name="tp", bufs=3) as tpool,
            tc.tile_pool(name="up", bufs=3) as upool,
            tc.tile_pool(name="sgp", bufs=2) as spool,
            tc.tile_pool(name="ps_l2", bufs=4, space="PSUM") as ps_l2,
            tc.tile_pool(name="ps_aux", bufs=2, space="PSUM") as ps_aux,
        ):
            # ---------------- constants / weights ----------------
            ident = cpool.tile([P, P], f32)
            make_identity(nc, ident)

            w1x = cpool.tile([P, H], f32r)  # [dx, h]
            nc.sync.dma_start(w1x[:], w1_d[:DX, :])
            w1y = cpool.tile([P, H], f32r)  # [dy, h]
            nc.sync.dma_start(w1y[:], w1_d[DX:, :])
            w2 = cpool.tile([P, HB, H], f32r)  # [p, kb, m]: W2[kb*P+p, m]
            nc.sync.dma_start(w2[:], w2_d.rearrange("(kb p) m -> p kb m", p=P))
            b1 = cpool.tile([P, HB], f32)
            nc.sync.dma_start(b1[:], b1_d.rearrange("(o p) -> p o", p=P))
            ascale = cpool.tile([P, HB], f32)
            nc.sync.dma_start(ascale[:], ascale_d.rearrange("(o p) -> p o", p=P))
            abias = cpool.tile([P, HB], f32)
            nc.sync.dma_start(abias[:], abias_d.rearrange("(o p) -> p o", p=P))
            s3 = cpool.tile([P, HB], f32)
            nc.sync.dma_start(s3[:], s3_d.rearrange("(o p) -> p o", p=P))
            b3r = cpool.tile([P, 1], f32)
            nc.sync.dma_start(b3r[:], b3r_d[:, None])
            ones = cpool.tile([P, 1], bf16)
            nc.vector.memset(ones[:], 1.0)

            # x natural layout, then PE-transpose to xT [dx, j]
            x_sb = cpool.tile([P, JB, DX], f32)  # x[jb*P+p, d]
            nc.sync.dma_start(x_sb[:], x_d.rearrange("(jb p) d -> p jb d", p=P))
            xT = cpool.tile([P, B], f32r)  # [dx, j]
            for jb in range(JB):
                ps_t = ps_aux.tile([P, P], f32, tag="tr")
                nc.tensor.transpose(ps_t[:], x_sb[:, jb, :], ident[:])
                nc.vector.tensor_copy(xT[:, jb * P : (jb + 1) * P], ps_t[:])

            ys_sb = cpool.tile([R, DY], f32)
            nc.sync.dma_start(ys_sb[:], ys_d[:, :])
            yT = cpool.tile([P, R], f32r)  # [dy, i]
            ps_t = ps_aux.tile([P, P], f32, tag="tr")
            nc.tensor.transpose(ps_t[:, :R], ys_sb[:], ident[:R, :R])
            nc.vector.tensor_copy(yT[:], ps_t[:, :R])

            # AT[h, j] = (x @ W1x).T ; CTb[h, i] = (ys @ W1y).T + b1[h]
            at = cpool.tile([P, HB, B], f32)
            ctb = cpool.tile([P, HB, R], f32)
            for hb in range(HB):
                hsl = slice(hb * P, (hb + 1) * P)
                ps_a = ps_l2.tile([P, B], f32, tag="l2")
                nc.tensor.matmul(ps_a[:], w1x[:, hsl], xT[:])
                nc.vector.tensor_copy(at[:, hb, :], ps_a[:])
                ps_c = ps_aux.tile([P, P], f32, tag="tr")
                nc.tensor.matmul(ps_c[:, :R], w1y[:, hsl], yT[:])
                nc.vector.tensor_scalar_add(
                    ctb[:, hb, :], ps_c[:, :R], scalar1=b1[:, hb : hb + 1]
                )

            # ---------------- main loop over the R y-rows ----------------
            # Row r's sign-sum + final matmul are emitted during row r+1's
            # layer-2 matmuls so the tensor engine never waits.
            t_live = {}
            u_live = {}
            sg_live = {}
            for it in range(nloop):
              for r in range(R + 2):
                if r < R:
                    # h1T = relu(AT + CTb[:, r])  (vector engine)
                    h1 = h1pool.tile([P, HB, B], f32r, tag="h1")
                    for hb in range(HB):
                        nc.vector.tensor_scalar(
                            out=h1[:, hb, :],
                            in0=at[:, hb, :],
                            scalar1=ctb[:, hb, r : r + 1],
                            scalar2=0.0,
                            op0=add,
                            op1=amax,
                        )
                    # z2T = W2.T @ h1T ; t = |w3| * relu(z2T + b2)
                    t = tpool.tile([P, HB, B], bf16, tag="t")
                    for mb in range(HB):
                        msl = slice(mb * P, (mb + 1) * P)
                        pl2 = ps_l2.tile([P, B], f32, tag="l2")
                        for kb in range(HB):
                            nc.tensor.matmul(
                                pl2[:],
                                w2[:, kb, msl],
                                h1[:, kb, :],
                                start=(kb == 0),
                                stop=(kb == HB - 1),
                            )
                        nc.scalar.activation(
                            t[:, mb, :],
                            pl2[:],
                            Relu,
                            bias=abias[:, mb : mb + 1],
                            scale=ascale[:, mb : mb + 1],
                        )
                    t_live[r] = t

                rr = r - 1
                if 0 <= rr < R:
                    # u = sum_kb sign3[:,kb] * t[:,kb,:] + b3/128  (vector)
                    tprev = t_live.pop(rr)
                    u = upool.tile([P, B], bf16, tag="u")
                    nc.vector.tensor_scalar(
                        out=u[:],
                        in0=tprev[:, 0, :],
                        scalar1=s3[:, 0:1],
                        scalar2=b3r[:],
                        op0=mult,
                        op1=add,
                    )
                    for kb in range(1, HB):
                        nc.vector.scalar_tensor_tensor(
                            out=u[:],
                            in0=tprev[:, kb, :],
                            scalar=s3[:, kb : kb + 1],
                            in1=u[:],
                            op0=mult,
                            op1=add,
                        )
                    u_live[rr] = u

                rq = r - 2
                if rq >= 0:
                    assert rq < R
                    # s[rq, :] = ones.T @ u  (single 512-cycle matmul)
                    uprev = u_live.pop(rq)
                    ps_s = ps_aux.tile([1, B], f32, tag="s")
                    nc.tensor.matmul(ps_s[:], ones[:], uprev[:])
                    g, gi = divmod(rq, GS)
                    if gi == 0:
                        sg_live[g] = spool.tile(
                            [1, GS, B], f32, tag="sg", name=f"sg_{it}_{g}"
                        )
                    nc.scalar.activation(sg_live[g][:, gi, :], ps_s[:], Copy)
                    if gi == GS - 1:
                        sg = sg_live.pop(g)
                        nc.sync.dma_start(out_d[g * GS : (g + 1) * GS, :], sg[:])

    nc.compile()
    return nc


def _get_nc(nloop=1):
    with _cache_lock:
        if nloop not in _cached_nc:
            _cached_nc[nloop] = _build_bass(nloop)
        return _cached_nc[nloop]


def prep_in_maps(inputs):
    x = np.ascontiguousarray(inputs["x"], dtype=np.float32)
    y = np.ascontiguousarray(inputs["y"], dtype=np.float32)
    b2 = np.asarray(inputs["b2"], dtype=np.float32)
    w3 = np.asarray(inputs["W3"], dtype=np.float32)[:, 0]
    b3 = np.asarray(inputs["b3"], dtype=np.float32)
    common = {
        "x": x,
        "w1": np.ascontiguousarray(inputs["W1"], dtype=np.float32),
        "b1": np.ascontiguousarray(inputs["b1"], dtype=np.float32),
        "w2": np.ascontiguousarray(inputs["W2"], dtype=np.float32),
        "ascale": np.ascontiguousarray(np.abs(w3)),
        "abias": np.ascontiguousarray(np.abs(w3) * b2),
        "s3": np.ascontiguousarray(np.sign(w3)),
        "b3r": np.full((P,), b3[0] / P, dtype=np.float32),
    }
    return [
        {**common, "ys": np.ascontiguousarray(y[d * R : (d + 1) * R])}
        for d in range(NCORES)
    ]


def run(inputs, trace=False, **run_kwargs):
    """Shard, run on 8 cores, gather. Returns (out [B,B] f32, BassKernelResults)."""
    from concourse import bass_utils

    nc = _get_nc()
    in_maps = prep_in_maps(inputs)
    res = bass_utils.run_bass_kernel_spmd(
        nc, in_maps, core_ids=list(range(NCORES)), trace=trace, **run_kwargs
    )
    s2 = np.concatenate([res.results[d]["s_slab"] for d in range(NCORES)], axis=0)
    return np.ascontiguousarray(s2.T), res


def kernel(**inputs) -> np.ndarray:
    # One retry: the axon-tunneled cores occasionally throw a transient
    # NRT_EXEC_UNIT_UNRECOVERABLE on the first touch after an idle period.
    try:
        out, _ = run(inputs, trace=False)
    except Exception:  # noqa: BLE001
        import time as _time

        _time.sleep(2.0)
        out, _ = run(inputs, trace=False)
    return out
